# revision 9
# baseline (speedup 1.0000x reference)
"""Linear-attention (ELU+1 feature map, causal multiplicative mask) TRN2 kernel.

Sharding: 8 cores = batch(2) x head-group(4).  Core c handles batch b=c//4 and
heads [g*8,(g+1)*8) where g=c%4 (512 of the 2048 feature dims).  Each core
computes its q/k/v projection slice, per-head quadratic causal attention, and a
partial out-projection over its 512 contraction dims; the host sums the 4
partials per batch (the out-proj all-reduce) and adds bo.

All matmuls run in float32r (TF32-like, full PE rate at N=512).  Activations /
normalization in fp32 on ACT/DVE.  elu(x)+1 == relu(x) + min(exp(x), 1).
Per-head sum over the 64 feature dims is a block-diagonal-ones matmul; the
reciprocal is broadcast back across partitions with a second ones matmul.
"""
import numpy as np
import concourse.bass as bass
import concourse.mybir as mybir
import concourse.tile as tile
from concourse import bacc
from concourse.bass_utils import run_bass_kernel_spmd
from concourse.alu_op_type import AluOpType

B, S, D = 2, 2048, 2048
H, HD = 32, 64
EPS = 1e-4
SC = HD ** -0.5  # 0.125
P = 128
SB = 512                 # s-block width
NSB = S // SB            # 4 s-blocks
KT = D // P              # 16 k tiles
MT = 4                   # 4 m-tiles of 128 per 512 local dims
F32 = mybir.dt.float32
F32R = mybir.dt.float32r
AF = mybir.ActivationFunctionType

_CACHE = {}


def _build():
    nc = bacc.Bacc()
    xT = nc.dram_tensor("xT", [D, S], F32R, kind="ExternalInput")
    wqT = nc.dram_tensor("wqT", [D, 512], F32R, kind="ExternalInput")
    wkT = nc.dram_tensor("wkT", [D, 512], F32R, kind="ExternalInput")
    wvT = nc.dram_tensor("wvT", [D, 512], F32R, kind="ExternalInput")
    woT = nc.dram_tensor("woT", [512, D], mybir.dt.bfloat16, kind="ExternalInput")
    bqs = nc.dram_tensor("bqs", [512, 1], F32, kind="ExternalInput")
    bks = nc.dram_tensor("bks", [512, 1], F32, kind="ExternalInput")
    bvrow = nc.dram_tensor("bvrow", [1, 512], F32R, kind="ExternalInput")
    masks = nc.dram_tensor("masks", [4, P, SB], mybir.dt.bfloat16, kind="ExternalInput")
    bd = nc.dram_tensor("bd", [P, 2], F32R, kind="ExternalInput")
    bdT = nc.dram_tensor("bdT", [2, P], F32R, kind="ExternalInput")
    ones1 = nc.dram_tensor("ones1", [1, P], F32R, kind="ExternalInput")
    outT = nc.dram_tensor("outT", [D, S], F32, kind="ExternalOutput")

    xT_r = xT.rearrange("(kt p) s -> p kt s", p=P)
    wqT_r = wqT.rearrange("(kt p) m -> p kt m", p=P)
    wkT_r = wkT.rearrange("(kt p) m -> p kt m", p=P)
    wvT_r = wvT.rearrange("(kt p) m -> p kt m", p=P)
    woT_r = woT.rearrange("(jt p) i -> p jt i", p=P)

    with tile.TileContext(nc) as tc:
        ctx_lp = nc.allow_low_precision(reason="f32r (tf32) matmul pipeline is intentional")
        ctx_lp.__enter__()
        with (
            tc.tile_pool(name="consts", bufs=1) as consts,
            tc.tile_pool(name="res", bufs=1) as res,
            tc.tile_pool(name="xblk", bufs=1) as xblk,
            tc.tile_pool(name="wtile", bufs=2) as wtile,
            tc.tile_pool(name="wotile", bufs=2) as wotile,
            tc.tile_pool(name="qn", bufs=5) as qn_pool,
            tc.tile_pool(name="elu", bufs=2) as elu_pool,
            tc.tile_pool(name="q1p", bufs=2) as q1_pool,
            tc.tile_pool(name="rqp", bufs=2) as rq_pool,
            tc.tile_pool(name="aop", bufs=4) as ao_pool,
            tc.tile_pool(name="atp", bufs=4) as at_pool,
            tc.tile_pool(name="outp", bufs=2) as out_pool,
            tc.tile_pool(name="ps", bufs=4, space="PSUM") as ps_pool,
            tc.tile_pool(name="pso", bufs=1, space="PSUM") as pso_pool,
            tc.tile_pool(name="pss", bufs=2, space="PSUM") as pss_pool,
        ):
            # ---- constants ----
            mask_t = []
            for r in range(4):
                mt_ = consts.tile([P, SB], mybir.dt.bfloat16, tag=f"mask{r}")
                nc.sync.dma_start(out=mt_, in_=masks[r])
                mask_t.append(mt_)
            bd_t = consts.tile([P, 2], F32R, tag="bd")
            nc.sync.dma_start(out=bd_t, in_=bd[:, :])
            bdT_t = consts.tile([2, P], F32R, tag="bdT")
            nc.sync.dma_start(out=bdT_t, in_=bdT[:, :])
            ones1_t = consts.tile([1, P], F32R, tag="ones1")
            nc.sync.dma_start(out=ones1_t, in_=ones1[:, :])
            bvrow_t = consts.tile([1, 512], F32R, tag="bvrow")
            nc.sync.dma_start(out=bvrow_t, in_=bvrow[:, :])
            bq_t, bk_t = [], []
            for m in range(MT):
                t = consts.tile([P, 1], F32, tag=f"bq{m}")
                nc.sync.dma_start(out=t, in_=bqs[m * P:(m + 1) * P, :])
                bq_t.append(t)
                t = consts.tile([P, 1], F32, tag=f"bk{m}")
                nc.sync.dma_start(out=t, in_=bks[m * P:(m + 1) * P, :])
                bk_t.append(t)
            epsq_t = consts.tile([2, 1], F32, tag="epsq")
            nc.vector.memset(epsq_t, EPS / SC)          # 8e-4
            epsk_t = consts.tile([2, 1], F32, tag="epsk")
            nc.vector.memset(epsk_t, EPS)               # 1e-4

            # ---- residents ----
            wv_s = res.tile([P, KT, 512], F32R, tag="wv")
            for q4 in range(4):
                nc.sync.dma_start(out=wv_s[:, q4 * 4:(q4 + 1) * 4, :],
                                  in_=wvT_r[:, q4 * 4:(q4 + 1) * 4, :])
            kn_t = [res.tile([P, S], F32R, tag=f"kn{m}", name=f"kn{m}") for m in range(MT)]
            v_s = res.tile([P, KT, 512], F32R, tag="v")

            for sj in range(NSB):
                s0 = sj * SB
                x_s = xblk.tile([P, KT, SB], F32R)
                for q4 in range(4):
                    nc.sync.dma_start(
                        out=x_s[:, q4 * 4:(q4 + 1) * 4, :],
                        in_=xT_r[:, q4 * 4:(q4 + 1) * 4, s0:s0 + SB])

                # ---- Q, K projections (feature-major [m, s]) + feature map ----
                qn_t = []
                for isq, (w_r, b_t, eps_t, scale) in enumerate(
                        ((wqT_r, bq_t, epsq_t, SC), (wkT_r, bk_t, epsk_t, 1.0))):
                    for m in range(MT):
                        w_s = wtile.tile([P, KT, P], F32R, tag="w")
                        for q4 in range(4):
                            nc.sync.dma_start(
                                out=w_s[:, q4 * 4:(q4 + 1) * 4, :],
                                in_=w_r[:, q4 * 4:(q4 + 1) * 4, m * P:(m + 1) * P])
                        ps = ps_pool.tile([P, SB], F32, tag="big")
                        for kt in range(KT):
                            nc.tensor.matmul(ps, w_s[:, kt, :], x_s[:, kt, :],
                                             start=(kt == 0), stop=(kt == KT - 1))
                        qr = elu_pool.tile([P, SB], F32, tag="qr")
                        nc.scalar.activation(out=qr, in_=ps, func=AF.Relu,
                                             bias=b_t[m], scale=scale)
                        qe = elu_pool.tile([P, SB], F32, tag="qe")
                        nc.scalar.activation(out=qe, in_=ps, func=AF.Exp,
                                             bias=b_t[m], scale=scale)
                        q1 = q1_pool.tile([P, SB], F32R)
                        nc.vector.scalar_tensor_tensor(
                            out=q1, in0=qe, scalar=1.0, in1=qr,
                            op0=AluOpType.min, op1=AluOpType.add)
                        pss = pss_pool.tile([2, SB], F32, tag="sum")
                        nc.tensor.matmul(pss, bd_t, q1, start=True, stop=True)
                        rt = rq_pool.tile([2, SB], F32, tag="rt")
                        nc.vector.tensor_scalar(
                            out=rt, in0=pss, scalar1=1.0 / scale,
                            scalar2=EPS / scale, op0=AluOpType.mult,
                            op1=AluOpType.add)
                        rq = rq_pool.tile([2, SB], F32R)
                        nc.vector.reciprocal(out=rq, in_=rt)
                        psb = ps_pool.tile([P, SB], F32, tag="big")
                        nc.tensor.matmul(psb, bdT_t, rq, start=True, stop=True)
                        if isq == 0:
                            dest = qn_pool.tile([P, SB], F32R)
                            qn_t.append(dest)
                        else:
                            dest = kn_t[m][:, s0:s0 + SB]
                        nc.vector.tensor_mul(dest, q1, psb)

                # ---- V projection (s-major [t, d]) ----
                for tsub in range(4):
                    ps = ps_pool.tile([P, 512], F32, tag="big")
                    for kt in range(KT):
                        nc.tensor.matmul(ps, x_s[:, kt, tsub * P:(tsub + 1) * P],
                                         wv_s[:, kt, :], start=(kt == 0), stop=False)
                    nc.tensor.matmul(ps, ones1_t, bvrow_t, start=False, stop=True)
                    nc.scalar.activation(out=v_s[:, sj * 4 + tsub, :], in_=ps,
                                         func=AF.Copy)

                # ---- attention, head pairs (A at partitions 0:64 -> PE row
                # groups 0-1, B at 64:128 -> groups 2-3: qk matmuls run
                # concurrently via auto-derived tile_position) ----
                ao_t = [ao_pool.tile([P, SB], mybir.dt.bfloat16, tag="ao", name="ao") for _ in range(MT)]
                nt = 4 * sj + 4
                for hp in range(4):
                    m = hp
                    qhA = qn_t[m][0:HD, :]
                    qhB = qn_t[m][HD:P, :]
                    ps_oA = pso_pool.tile([HD, SB], F32, tag="poA")
                    ps_oB = pso_pool.tile([HD, SB], F32, tag="poB")
                    for ti in range(nt):
                        ps_aA = ps_pool.tile([P, SB], F32, tag="big")
                        ps_aB = ps_pool.tile([P, SB], F32, tag="big")
                        nc.tensor.matmul(ps_aA,
                                         kn_t[m][0:HD, ti * P:(ti + 1) * P],
                                         qhA, start=True, stop=True)
                        nc.tensor.matmul(ps_aB,
                                         kn_t[m][HD:P, ti * P:(ti + 1) * P],
                                         qhB, start=True, stop=True)
                        a_tA = at_pool.tile([P, SB], F32R, tag="at")
                        a_tB = at_pool.tile([P, SB], F32R, tag="at")
                        r = ti - 4 * sj
                        if r >= 0:
                            nc.vector.tensor_mul(a_tA, ps_aA, mask_t[r])
                            nc.vector.tensor_mul(a_tB, ps_aB, mask_t[r])
                        else:
                            nc.vector.tensor_copy(out=a_tA, in_=ps_aA)
                            nc.vector.tensor_copy(out=a_tB, in_=ps_aB)
                        nc.tensor.matmul(ps_oA, v_s[:, ti, (2 * hp) * HD:(2 * hp + 1) * HD],
                                         a_tA, start=(ti == 0), stop=(ti == nt - 1))
                        nc.tensor.matmul(ps_oB, v_s[:, ti, (2 * hp + 1) * HD:(2 * hp + 2) * HD],
                                         a_tB, start=(ti == 0), stop=(ti == nt - 1))
                    nc.scalar.activation(out=ao_t[m][0:HD, :], in_=ps_oA,
                                         func=AF.Copy)
                    nc.scalar.activation(out=ao_t[m][HD:P, :], in_=ps_oB,
                                         func=AF.Copy)

                # ---- partial out-projection (feature-major [i, s]) ----
                for it in range(KT):
                    wo_s = wotile.tile([P, MT, P], mybir.dt.bfloat16, tag="wo")
                    nc.sync.dma_start(out=wo_s, in_=woT_r[:, :, it * P:(it + 1) * P])
                    ps = ps_pool.tile([P, SB], F32, tag="big")
                    for jt in range(MT):
                        nc.tensor.matmul(ps, wo_s[:, jt, :], ao_t[jt],
                                         start=(jt == 0), stop=(jt == MT - 1))
                    o_t = out_pool.tile([P, SB], F32, tag="ot")
                    nc.vector.tensor_copy(out=o_t, in_=ps)
                    nc.sync.dma_start(out=outT[it * P:(it + 1) * P, s0:s0 + SB],
                                      in_=o_t)
    nc.compile()
    return nc


def _in_maps(hidden_states, wq, bq, wk, bk, wv, bv, wo):
    import ml_dtypes
    f32 = np.float32
    mask_np = np.zeros((4, P, SB), ml_dtypes.bfloat16)
    for r in range(4):
        p = np.arange(P)[:, None] + r * P
        f = np.arange(SB)[None, :]
        mask_np[r] = (p <= f).astype(ml_dtypes.bfloat16)
    bd_np = np.zeros((P, 2), f32)
    bd_np[:HD, 0] = 1.0
    bd_np[HD:, 1] = 1.0
    bdT_np = bd_np.T.copy()
    ones1_np = np.ones((1, P), f32)
    ins = []
    for c in range(8):
        b, g = c // 4, c % 4
        cols = slice(g * 512, (g + 1) * 512)
        ins.append({
            "xT": np.ascontiguousarray(hidden_states[b].T),
            "wqT": np.ascontiguousarray(wq[cols, :].T),
            "wkT": np.ascontiguousarray(wk[cols, :].T),
            "wvT": np.ascontiguousarray(wv[cols, :].T),
            "woT": np.ascontiguousarray(wo[:, cols].T).astype(ml_dtypes.bfloat16),
            "bqs": (bq[cols] * SC).reshape(512, 1).astype(f32),
            "bks": bk[cols].reshape(512, 1).astype(f32),
            "bvrow": bv[cols].reshape(1, 512).astype(f32),
            "masks": mask_np,
            "bd": bd_np,
            "bdT": bdT_np,
            "ones1": ones1_np,
        })
    return ins


def _run(inputs, trace=False):
    hs = np.asarray(inputs["hidden_states"], np.float32)
    if "nc" not in _CACHE:
        _CACHE["nc"] = _build()
    nc = _CACHE["nc"]
    ins = _in_maps(hs, np.asarray(inputs["wq"], np.float32),
                   np.asarray(inputs["bq"], np.float32),
                   np.asarray(inputs["wk"], np.float32),
                   np.asarray(inputs["bk"], np.float32),
                   np.asarray(inputs["wv"], np.float32),
                   np.asarray(inputs["bv"], np.float32),
                   np.asarray(inputs["wo"], np.float32))
    res = run_bass_kernel_spmd(nc, ins, core_ids=list(range(8)), trace=trace)
    bo = np.asarray(inputs["bo"], np.float32)
    out = np.zeros((B, S, D), np.float32)
    for b in range(B):
        acc = res.results[4 * b]["outT"].copy()
        for g in range(1, 4):
            acc += res.results[4 * b + g]["outT"]
        out[b] = acc.T + bo
    return out, getattr(res, "exec_time_ns", None)


def kernel(**inputs):
    return _run(inputs)[0]


# revision 10
# speedup vs baseline: 1.1271x; 1.1271x over previous
"""Linear-attention (ELU+1 feature map, causal multiplicative mask) TRN2 kernel.

Sharding: 8 cores = batch(2) x head-group(4).  Core c handles batch b=c//4 and
heads [g*8,(g+1)*8) where g=c%4 (512 of the 2048 feature dims).  Each core
computes its q/k/v projection slice, per-head quadratic causal attention, and a
partial out-projection over its 512 contraction dims; the host sums the 4
partials per batch (the out-proj all-reduce) and adds bo.

All matmuls run in float32r (TF32-like, full PE rate at N=512).  Activations /
normalization in fp32 on ACT/DVE.  elu(x)+1 == relu(x) + min(exp(x), 1).
Per-head sum over the 64 feature dims is a block-diagonal-ones matmul; the
reciprocal is broadcast back across partitions with a second ones matmul.
"""
import numpy as np
import concourse.bass as bass
import concourse.mybir as mybir
import concourse.tile as tile
from concourse import bacc
from concourse.bass_utils import run_bass_kernel_spmd
from concourse.alu_op_type import AluOpType

B, S, D = 2, 2048, 2048
H, HD = 32, 64
EPS = 1e-4
SC = HD ** -0.5  # 0.125
P = 128
SB = 512                 # s-block width
NSB = S // SB            # 4 s-blocks
KT = D // P              # 16 k tiles
MT = 4                   # 4 m-tiles of 128 per 512 local dims
F32 = mybir.dt.float32
F32R = mybir.dt.float32r
AF = mybir.ActivationFunctionType

_CACHE = {}


def _build():
    nc = bacc.Bacc()
    xT = nc.dram_tensor("xT", [D, S], F32R, kind="ExternalInput")
    wqT = nc.dram_tensor("wqT", [D, 512], F32R, kind="ExternalInput")
    wkT = nc.dram_tensor("wkT", [D, 512], F32R, kind="ExternalInput")
    wvT = nc.dram_tensor("wvT", [D, 512], F32R, kind="ExternalInput")
    woT = nc.dram_tensor("woT", [512, D], mybir.dt.bfloat16, kind="ExternalInput")
    bqs = nc.dram_tensor("bqs", [512, 1], F32, kind="ExternalInput")
    bks = nc.dram_tensor("bks", [512, 1], F32, kind="ExternalInput")
    bvrow = nc.dram_tensor("bvrow", [1, 512], F32R, kind="ExternalInput")
    masks = nc.dram_tensor("masks", [4, P, SB], mybir.dt.bfloat16, kind="ExternalInput")
    bd = nc.dram_tensor("bd", [P, 2], F32R, kind="ExternalInput")
    bdT = nc.dram_tensor("bdT", [2, P], F32R, kind="ExternalInput")
    ones1 = nc.dram_tensor("ones1", [1, P], F32R, kind="ExternalInput")
    outT = nc.dram_tensor("outT", [D, S], F32, kind="ExternalOutput")

    xT_r = xT.rearrange("(kt p) s -> p kt s", p=P)
    wqT_r = wqT.rearrange("(kt p) m -> p kt m", p=P)
    wkT_r = wkT.rearrange("(kt p) m -> p kt m", p=P)
    wvT_r = wvT.rearrange("(kt p) m -> p kt m", p=P)
    woT_r = woT.rearrange("(jt p) i -> p jt i", p=P)

    with tile.TileContext(nc) as tc:
        ctx_lp = nc.allow_low_precision(reason="f32r (tf32) matmul pipeline is intentional")
        ctx_lp.__enter__()
        with (
            tc.tile_pool(name="consts", bufs=1) as consts,
            tc.tile_pool(name="res", bufs=1) as res,
            tc.tile_pool(name="xblk", bufs=1) as xblk,
            tc.tile_pool(name="wtile", bufs=2) as wtile,
            tc.tile_pool(name="wotile", bufs=2) as wotile,
            tc.tile_pool(name="qn", bufs=5) as qn_pool,
            tc.tile_pool(name="elu", bufs=2) as elu_pool,
            tc.tile_pool(name="q1p", bufs=2) as q1_pool,
            tc.tile_pool(name="rqp", bufs=2) as rq_pool,
            tc.tile_pool(name="aop", bufs=4) as ao_pool,
            tc.tile_pool(name="atp", bufs=4) as at_pool,
            tc.tile_pool(name="outp", bufs=2) as out_pool,
            tc.tile_pool(name="ps", bufs=4, space="PSUM") as ps_pool,
            tc.tile_pool(name="pso", bufs=1, space="PSUM") as pso_pool,
            tc.tile_pool(name="pss", bufs=2, space="PSUM") as pss_pool,
        ):
            # ---- constants ----
            mask_t = []
            for r in range(4):
                mt_ = consts.tile([P, SB], mybir.dt.bfloat16, tag=f"mask{r}")
                nc.sync.dma_start(out=mt_, in_=masks[r])
                mask_t.append(mt_)
            bd_t = consts.tile([P, 2], F32R, tag="bd")
            nc.sync.dma_start(out=bd_t, in_=bd[:, :])
            bdT_t = consts.tile([2, P], F32R, tag="bdT")
            nc.sync.dma_start(out=bdT_t, in_=bdT[:, :])
            ones1_t = consts.tile([1, P], F32R, tag="ones1")
            nc.sync.dma_start(out=ones1_t, in_=ones1[:, :])
            bvrow_t = consts.tile([1, 512], F32R, tag="bvrow")
            nc.sync.dma_start(out=bvrow_t, in_=bvrow[:, :])
            bq_t, bk_t = [], []
            for m in range(MT):
                t = consts.tile([P, 1], F32, tag=f"bq{m}")
                nc.sync.dma_start(out=t, in_=bqs[m * P:(m + 1) * P, :])
                bq_t.append(t)
                t = consts.tile([P, 1], F32, tag=f"bk{m}")
                nc.sync.dma_start(out=t, in_=bks[m * P:(m + 1) * P, :])
                bk_t.append(t)
            epsq_t = consts.tile([2, 1], F32, tag="epsq")
            nc.vector.memset(epsq_t, EPS / SC)          # 8e-4
            epsk_t = consts.tile([2, 1], F32, tag="epsk")
            nc.vector.memset(epsk_t, EPS)               # 1e-4

            # ---- residents ----
            wv_s = res.tile([P, KT, 512], F32R, tag="wv")
            for q4 in range(4):
                nc.sync.dma_start(out=wv_s[:, q4 * 4:(q4 + 1) * 4, :],
                                  in_=wvT_r[:, q4 * 4:(q4 + 1) * 4, :])
            kn_t = [res.tile([P, S], F32R, tag=f"kn{m}", name=f"kn{m}") for m in range(MT)]
            v_s = res.tile([P, KT, 512], F32R, tag="v")

            for sj in range(NSB):
                s0 = sj * SB
                x_s = xblk.tile([P, KT, SB], F32R)
                for q4 in range(4):
                    nc.sync.dma_start(
                        out=x_s[:, q4 * 4:(q4 + 1) * 4, :],
                        in_=xT_r[:, q4 * 4:(q4 + 1) * 4, s0:s0 + SB])

                # ---- Q, K projections (feature-major [m, s]) + feature map ----
                qn_t = []
                for isq, (w_r, b_t, eps_t, scale) in enumerate(
                        ((wqT_r, bq_t, epsq_t, SC), (wkT_r, bk_t, epsk_t, 1.0))):
                    for m in range(MT):
                        w_s = wtile.tile([P, KT, P], F32R, tag="w")
                        for q4 in range(4):
                            nc.sync.dma_start(
                                out=w_s[:, q4 * 4:(q4 + 1) * 4, :],
                                in_=w_r[:, q4 * 4:(q4 + 1) * 4, m * P:(m + 1) * P])
                        ps = ps_pool.tile([P, SB], F32, tag="big")
                        for kt in range(KT):
                            nc.tensor.matmul(ps, w_s[:, kt, :], x_s[:, kt, :],
                                             start=(kt == 0), stop=(kt == KT - 1))
                        qr = elu_pool.tile([P, SB], F32, tag="qr")
                        nc.scalar.activation(out=qr, in_=ps, func=AF.Relu,
                                             bias=b_t[m], scale=scale)
                        qe = elu_pool.tile([P, SB], F32, tag="qe")
                        nc.scalar.activation(out=qe, in_=ps, func=AF.Exp,
                                             bias=b_t[m], scale=scale)
                        q1 = q1_pool.tile([P, SB], F32R)
                        nc.vector.scalar_tensor_tensor(
                            out=q1, in0=qe, scalar=1.0, in1=qr,
                            op0=AluOpType.min, op1=AluOpType.add)
                        pss = pss_pool.tile([2, SB], F32, tag="sum")
                        nc.tensor.matmul(pss, bd_t, q1, start=True, stop=True)
                        rt = rq_pool.tile([2, SB], F32, tag="rt")
                        nc.vector.tensor_scalar(
                            out=rt, in0=pss, scalar1=1.0 / scale,
                            scalar2=EPS / scale, op0=AluOpType.mult,
                            op1=AluOpType.add)
                        rq = rq_pool.tile([2, SB], F32R)
                        nc.vector.reciprocal(out=rq, in_=rt)
                        psb = ps_pool.tile([P, SB], F32, tag="big")
                        nc.tensor.matmul(psb, bdT_t, rq, start=True, stop=True)
                        if isq == 0:
                            dest = qn_pool.tile([P, SB], F32R)
                            qn_t.append(dest)
                        else:
                            dest = kn_t[m][:, s0:s0 + SB]
                        nc.vector.tensor_mul(dest, q1, psb)

                # ---- V projection (s-major [t, d]) ----
                for tsub in range(4):
                    ps = ps_pool.tile([P, 512], F32, tag="big")
                    for kt in range(KT):
                        nc.tensor.matmul(ps, x_s[:, kt, tsub * P:(tsub + 1) * P],
                                         wv_s[:, kt, :], start=(kt == 0), stop=False)
                    nc.tensor.matmul(ps, ones1_t, bvrow_t, start=False, stop=True)
                    nc.scalar.activation(out=v_s[:, sj * 4 + tsub, :], in_=ps,
                                         func=AF.Copy)

                # ---- attention, head pairs (A at partitions 0:64 -> PE row
                # groups 0-1, B at 64:128 -> groups 2-3: qk matmuls run
                # concurrently via auto-derived tile_position) ----
                ao_t = [ao_pool.tile([P, SB], mybir.dt.bfloat16, tag="ao", name="ao") for _ in range(MT)]
                nt = 4 * sj + 4
                for hp in range(4):
                    m = hp
                    qhA = qn_t[m][0:HD, :]
                    qhB = qn_t[m][HD:P, :]
                    ps_oA = pso_pool.tile([HD, SB], F32, tag="poA")
                    ps_oB = pso_pool.tile([HD, SB], F32, tag="poB")
                    for ti in range(nt):
                        ps_aA = ps_pool.tile([P, SB], F32, tag="big")
                        ps_aB = ps_pool.tile([P, SB], F32, tag="big")
                        nc.tensor.matmul(ps_aA,
                                         kn_t[m][0:HD, ti * P:(ti + 1) * P],
                                         qhA, start=True, stop=True)
                        nc.tensor.matmul(ps_aB,
                                         kn_t[m][HD:P, ti * P:(ti + 1) * P],
                                         qhB, start=True, stop=True)
                        a_tA = at_pool.tile([P, SB], F32R, tag="at")
                        a_tB = at_pool.tile([P, SB], F32R, tag="at")
                        r = ti - 4 * sj
                        if r >= 0:
                            nc.vector.tensor_mul(a_tA, ps_aA, mask_t[r])
                            nc.vector.tensor_mul(a_tB, ps_aB, mask_t[r])
                        else:
                            nc.vector.tensor_copy(out=a_tA, in_=ps_aA)
                            nc.vector.tensor_copy(out=a_tB, in_=ps_aB)
                        nc.tensor.matmul(ps_oA, v_s[:, ti, (2 * hp) * HD:(2 * hp + 1) * HD],
                                         a_tA, start=(ti == 0), stop=(ti == nt - 1))
                        nc.tensor.matmul(ps_oB, v_s[:, ti, (2 * hp + 1) * HD:(2 * hp + 2) * HD],
                                         a_tB, start=(ti == 0), stop=(ti == nt - 1))
                    nc.scalar.activation(out=ao_t[m][0:HD, :], in_=ps_oA,
                                         func=AF.Copy)
                    nc.scalar.activation(out=ao_t[m][HD:P, :], in_=ps_oB,
                                         func=AF.Copy)

                # ---- partial out-projection (feature-major [i, s]) ----
                for it in range(KT):
                    wo_s = wotile.tile([P, MT, P], mybir.dt.bfloat16, tag="wo")
                    nc.sync.dma_start(out=wo_s, in_=woT_r[:, :, it * P:(it + 1) * P])
                    ps = ps_pool.tile([P, SB], F32, tag="big")
                    for jt in range(MT):
                        nc.tensor.matmul(ps, wo_s[:, jt, :], ao_t[jt],
                                         start=(jt == 0), stop=(jt == MT - 1))
                    o_t = out_pool.tile([P, SB], F32, tag="ot")
                    nc.vector.tensor_copy(out=o_t, in_=ps)
                    nc.sync.dma_start(out=outT[it * P:(it + 1) * P, s0:s0 + SB],
                                      in_=o_t)
    nc.compile()
    return nc


def _in_maps(hidden_states, wq, bq, wk, bk, wv, bv, wo):
    import ml_dtypes
    f32 = np.float32
    mask_np = np.zeros((4, P, SB), ml_dtypes.bfloat16)
    for r in range(4):
        p = np.arange(P)[:, None] + r * P
        f = np.arange(SB)[None, :]
        mask_np[r] = (p <= f).astype(ml_dtypes.bfloat16)
    bd_np = np.zeros((P, 2), f32)
    bd_np[:HD, 0] = 1.0
    bd_np[HD:, 1] = 1.0
    bdT_np = bd_np.T.copy()
    ones1_np = np.ones((1, P), f32)
    xTb = [np.ascontiguousarray(hidden_states[b].T) for b in range(B)]
    ins = []
    for c in range(8):
        b, g = c // 4, c % 4
        cols = slice(g * 512, (g + 1) * 512)
        ins.append({
            "xT": xTb[b],
            "wqT": np.ascontiguousarray(wq[cols, :].T),
            "wkT": np.ascontiguousarray(wk[cols, :].T),
            "wvT": np.ascontiguousarray(wv[cols, :].T),
            "woT": np.ascontiguousarray(wo[:, cols].T).astype(ml_dtypes.bfloat16),
            "bqs": (bq[cols] * SC).reshape(512, 1).astype(f32),
            "bks": bk[cols].reshape(512, 1).astype(f32),
            "bvrow": bv[cols].reshape(1, 512).astype(f32),
            "masks": mask_np,
            "bd": bd_np,
            "bdT": bdT_np,
            "ones1": ones1_np,
        })
    return ins


def _run(inputs, trace=False):
    hs = np.asarray(inputs["hidden_states"], np.float32)
    if "nc" not in _CACHE:
        _CACHE["nc"] = _build()
    nc = _CACHE["nc"]
    ins = _in_maps(hs, np.asarray(inputs["wq"], np.float32),
                   np.asarray(inputs["bq"], np.float32),
                   np.asarray(inputs["wk"], np.float32),
                   np.asarray(inputs["bk"], np.float32),
                   np.asarray(inputs["wv"], np.float32),
                   np.asarray(inputs["bv"], np.float32),
                   np.asarray(inputs["wo"], np.float32))
    res = run_bass_kernel_spmd(nc, ins, core_ids=list(range(8)), trace=trace)
    bo = np.asarray(inputs["bo"], np.float32)
    out = np.zeros((B, S, D), np.float32)
    for b in range(B):
        acc = res.results[4 * b]["outT"].copy()
        for g in range(1, 4):
            acc += res.results[4 * b + g]["outT"]
        out[b] = acc.T + bo
    return out, getattr(res, "exec_time_ns", None)


def kernel(**inputs):
    return _run(inputs)[0]
